# revision 2
# baseline (speedup 1.0000x reference)
"""Trainium2 Bass kernel for a dense transformer block (nn_Block_120259084502).

Contract: kernel(**inputs) takes the FULL unsharded inputs (numpy, fp32) and
returns the FULL output [4, 2048, 1024] fp32. Internally shards across 8
NeuronCores: core c handles batch c//2, query-token half c%2. Each core
receives its batch's full 2048 tokens (rolled so its own 1024 query tokens
come first) and computes K/V for all of them locally, so no collectives are
needed (attention context = full batch; softmax is order-invariant so the
roll is harmless).

Attention path (QKV, scores, exp*V, proj) runs in fp8 e4m3 with DoubleRow
perf mode (2x matmul rate) where the contraction allows; the MLP stays bf16
(fp8 there would exceed the error budget). PSUM accumulation is fp32
everywhere; LayerNorm statistics and residual adds stay fp32. Host-side
scale folding keeps every fp8 tensor in e4m3's normal range:
  wq,wk scaled x8 (the 1/sqrt(D) and 1/64 compensations fold into the Exp
  activation's scale), wv and proj_w scaled x16 (compensated by a 1/256
  scale on the proj PSUM copy-out).
"""

import numpy as np
import ml_dtypes

import concourse.bacc as bacc
import concourse.tile as tile
from concourse import mybir
from concourse.bass_utils import run_bass_kernel_spmd
from concourse.masks import make_identity

bf16 = mybir.dt.bfloat16
f8 = mybir.dt.float8e4
f32 = mybir.dt.float32
AF = mybir.ActivationFunctionType
ALU = mybir.AluOpType
DR = mybir.MatmulPerfMode.DoubleRow

P = 128
B, T, E, H, D = 4, 2048, 1024, 16, 64
F = 4 * E                    # 4096 MLP hidden
TQ = T // 2                  # 1024 own query tokens per core
NE = E // P                  # 8 e-chunks
NE2 = NE // 2                # 4 e-chunk pairs (DoubleRow)
NPAIR = H // 2               # 8 head pairs
NST = T // P                 # 16 context-token tiles
NSP = NST // 2               # 8 context-token tile pairs
NTS = TQ // P                # 8 own-token tiles
NF = F // P                  # 32 f-chunks
VW = D + 1                   # per-head V width incl. ones column
LN_EPS = 1e-5
QS = 8.0                     # host scale on wq,wk
VS = 16.0                    # host scale on wv
PS = 16.0                    # host scale on proj_w
EXP_SCALE = (D ** -0.5) / (QS * QS)
PROJ_OUT_SCALE = 1.0 / (VS * PS)

_BUILD_CACHE = {}


class _Ctx:
    """Shared build state passed between phase emitters."""
    pass


def _emit_ln(g, xt, out_lp):
    nc = g.nc
    st = g.stat.tile([P, 2, nc.vector.BN_STATS_DIM], f32, name="bnst")
    xv = xt.rearrange("p (s g) -> p s g", s=2)
    nc.vector.bn_stats(out=st[:, 0, :], in_=xv[:, 0, :])
    nc.vector.bn_stats(out=st[:, 1, :], in_=xv[:, 1, :])
    mv = g.stat.tile([P, nc.vector.BN_AGGR_DIM], f32, name="bnmv")
    nc.vector.bn_aggr(out=mv, in_=st)
    rstd = g.stat.tile([P, 1], f32, name="bnrs")
    nc.scalar.activation(out=rstd, in_=mv[:, 1:2], func=AF.Sqrt, bias=g.eps_t)
    nc.vector.reciprocal(out=rstd, in_=rstd)
    nc.vector.tensor_scalar(
        out=out_lp, in0=xt, scalar1=mv[:, 0:1], scalar2=rstd,
        op0=ALU.subtract, op1=ALU.mult,
    )


def _emit_consts(g):
    nc, consts = g.nc, g.consts
    g.ident8 = consts.tile([P, P], f8, name="ident8")
    make_identity(nc, g.ident8)
    g.ident = consts.tile([P, P], bf16, name="ident")
    make_identity(nc, g.ident)
    g.eps_t = consts.tile([P, 1], f32, name="eps")
    nc.vector.memset(g.eps_t, LN_EPS)
    g.ub_sb = consts.tile([P, NF], f32, name="ubsb")
    nc.sync.dma_start(out=g.ub_sb, in_=g.ub_d[:, :])
    if g.has_qb:
        g.qb_sb = consts.tile([P, NPAIR], f32, name="qbsb")
        nc.sync.dma_start(out=g.qb_sb, in_=g.qb_d[:, :])
        g.kb_sb = consts.tile([P, NPAIR], f32, name="kbsb")
        nc.sync.dma_start(out=g.kb_sb, in_=g.kb_d[:, :])
        g.vb_bc = consts.tile([P, E], bf16, name="vbbc")
        nc.gpsimd.dma_start(
            out=g.vb_bc, in_=g.vbrow_d.ap()[0:1, :].partition_broadcast(P)[:, 0, :]
        )
    if g.has_pb:
        g.pb_bc = consts.tile([P, E], f32, name="pbbc")
        nc.gpsimd.dma_start(
            out=g.pb_bc, in_=g.pbrow_d.ap()[0:1, :].partition_broadcast(P)[:, 0, :]
        )
    if g.has_db:
        g.db_bc = consts.tile([P, E], f32, name="dbbc")
        nc.gpsimd.dma_start(
            out=g.db_bc, in_=g.dbrow_d.ap()[0:1, :].partition_broadcast(P)[:, 0, :]
        )


def _emit_ln1_transpose(g, xkp, tps):
    """Load x, LN1, PE-transpose h into e-major hT (fp8)."""
    nc = g.nc
    for i in range(NST):
        xt = xkp.tile([P, E], f32, name="xk")
        nc.sync.dma_start(out=xt, in_=g.xkv_d[i * P:(i + 1) * P, :])
        ht = g.hp.tile([P, E], f8, name="h")
        _emit_ln(g, xt, ht)
        for c in range(NE):
            tp = tps.tile([P, P], f8, name="tp")
            nc.tensor.transpose(tp, ht[:, c * P:(c + 1) * P], g.ident8)
            nc.vector.tensor_copy(out=g.hT[:, c, i * P:(i + 1) * P], in_=tp)


def _emit_v(g, wvp, vps):
    """V (x16) in natural [s, d] layout, paired s-tiles, ones col per head."""
    nc = g.nc
    wv_sb = wvp.tile([P, NE2, 2, E], f8, name="wv")
    for c2 in range(NE2):
        nc.sync.dma_start(out=wv_sb[:, c2], in_=g.wv_d[c2])
    for sp in range(NSP):
        nc.gpsimd.dma_start(
            out=g.va[sp],
            in_=g.vrow_d.ap()[0:1, :].partition_broadcast(P)[:, 0, :],
        )
        vav = g.va[sp].rearrange("p (i h c) -> p i h c", i=2, c=VW)
        for i in range(2):
            s = 2 * sp + i
            pv = [vps.tile([P, 512], f32, name=f"pv{j}") for j in range(2)]
            for c2 in range(NE2):
                for j in range(2):
                    nc.tensor.matmul(
                        pv[j], g.hT[:, 2 * c2:2 * c2 + 2, s * P:(s + 1) * P],
                        wv_sb[:, c2, :, j * 512:(j + 1) * 512],
                        start=(c2 == 0), stop=(c2 == NE2 - 1), perf_mode=DR,
                    )
            for j in range(2):
                dst = vav[:, i, j * 8:(j + 1) * 8, 0:D]
                src = pv[j].rearrange("p (h d) -> p h d", d=D)
                if g.has_qb:
                    vb_view = g.vb_bc.rearrange("p (h d) -> p h d", d=D)[
                        :, j * 8:(j + 1) * 8, :
                    ]
                    nc.vector.tensor_add(out=dst, in0=src, in1=vb_view)
                else:
                    nc.vector.tensor_copy(out=dst, in_=src)


def _emit_qkt_pair(g, p, qt, kt, wqkp, qkps):
    """Q^T and K^T for head pair p: [128 (2 heads x 64d), tokens], fp8."""
    nc = g.nc
    psq = [qkps.tile([P, 512], f32, name=f"ps{j}") for j in range(2)]
    for c2 in range(NE2):
        wsl = wqkp.tile([P, 2, P], f8, name="wsl")
        nc.sync.dma_start(out=wsl, in_=g.wq_d[c2, p])
        for j in range(2):
            nc.tensor.matmul(
                psq[j], wsl, g.hT[:, 2 * c2:2 * c2 + 2, j * 512:(j + 1) * 512],
                start=(c2 == 0), stop=(c2 == NE2 - 1), perf_mode=DR,
            )
    for j in range(2):
        dst = qt[:, j * 512:(j + 1) * 512]
        if g.has_qb:
            nc.vector.tensor_scalar(
                out=dst, in0=psq[j], scalar1=g.qb_sb[:, p:p + 1], op0=ALU.add
            )
        else:
            nc.vector.tensor_copy(out=dst, in_=psq[j])
    for sh in range(2):
        psk = [qkps.tile([P, 512], f32, name=f"ps{j}") for j in range(2)]
        for c2 in range(NE2):
            wsl = wqkp.tile([P, 2, P], f8, name="wsl")
            nc.sync.dma_start(out=wsl, in_=g.wk_d[c2, p])
            for j in range(2):
                s0 = (sh * 2 + j) * 512
                nc.tensor.matmul(
                    psk[j], wsl, g.hT[:, 2 * c2:2 * c2 + 2, s0:s0 + 512],
                    start=(c2 == 0), stop=(c2 == NE2 - 1), perf_mode=DR,
                )
        for j in range(2):
            s0 = (sh * 2 + j) * 512
            dst = kt[:, s0:s0 + 512]
            if g.has_qb:
                nc.vector.tensor_scalar(
                    out=dst, in0=psk[j], scalar1=g.kb_sb[:, p:p + 1], op0=ALU.add
                )
            else:
                nc.vector.tensor_copy(out=dst, in_=psk[j])


def _emit_attn_pair(g, p, qt, kt, ptp, smp, scps, atps):
    """Scores (transposed), exp (fp8), attn^T via DoubleRow over s-pairs,
    softmax denom from ones column, normalize -> catT."""
    nc = g.nc
    for th in range(2):
        tcols = slice(th * 512, (th + 1) * 512)
        at0 = atps.tile([D + 1, 512], f32, name="ps0")
        at1 = atps.tile([D + 1, 512], f32, name="ps1")
        for sp in range(NSP):
            pt0 = ptp.tile([P, 2, 512], f8, name="pt0")
            pt1 = ptp.tile([P, 2, 512], f8, name="pt1")
            for i in range(2):
                s = 2 * sp + i
                scols = slice(s * P, (s + 1) * P)
                sc0 = scps.tile([P, 512], f32, name="sc0")
                sc1 = scps.tile([P, 512], f32, name="sc1")
                # S^T[s,t] = (K^T slice).T @ Q^T slice; the two heads live on
                # row-groups 0-63 / 64-127 so the matmuls pack concurrently.
                nc.tensor.matmul(sc0, kt[0:D, scols], qt[0:D, tcols],
                                 start=True, stop=True)
                nc.tensor.matmul(sc1, kt[D:2 * D, scols], qt[D:2 * D, tcols],
                                 start=True, stop=True)
                nc.scalar.activation(out=pt0[:, i, :], in_=sc0, func=AF.Exp,
                                     scale=EXP_SCALE, bias=-2.0)
                nc.scalar.activation(out=pt1[:, i, :], in_=sc1, func=AF.Exp,
                                     scale=EXP_SCALE, bias=-2.0)
            nc.tensor.matmul(
                at0, g.va[sp][:, :, (2 * p) * VW:(2 * p) * VW + VW], pt0,
                start=(sp == 0), stop=(sp == NSP - 1), perf_mode=DR,
            )
            nc.tensor.matmul(
                at1, g.va[sp][:, :, (2 * p + 1) * VW:(2 * p + 1) * VW + VW], pt1,
                start=(sp == 0), stop=(sp == NSP - 1), perf_mode=DR,
            )
        se0 = smp.tile([1, 512], f32, name="se0")
        se1 = smp.tile([1, 512], f32, name="se1")
        nc.vector.reciprocal(out=se0, in_=at0[D:D + 1, :])
        nc.vector.reciprocal(out=se1, in_=at1[D:D + 1, :])
        rb0 = smp.tile([D, 512], f32, name="rb0")
        rb1 = smp.tile([D, 512], f32, name="rb1")
        nc.gpsimd.partition_broadcast(rb0, se0)
        nc.gpsimd.partition_broadcast(rb1, se1)
        nc.vector.tensor_mul(out=g.catT[0:D, p, tcols], in0=at0[0:D, :], in1=rb0)
        nc.vector.tensor_mul(out=g.catT[D:2 * D, p, tcols], in0=at1[0:D, :],
                             in1=rb1)


def _emit_proj_ln2(g, uwp, xq2p, h2p, pps, t2ps):
    nc = g.nc
    pw_sb = g.pw_sb
    g.uw_sb = []
    for c in range(NE):  # prefetch MLP up-weights while proj runs
        w = uwp.tile([P, F], bf16, name=f"uw{c}")
        nc.sync.dma_start(out=w, in_=g.uw_d[c])
        g.uw_sb.append(w)
    for ts in range(NTS):
        trows = slice(ts * P, (ts + 1) * P)
        xres = xq2p.tile([P, E], f32, name="xres")
        nc.sync.dma_start(out=xres, in_=g.xkv_d[ts * P:(ts + 1) * P, :])
        psy = [pps.tile([P, 512], f32, name=f"py{j}") for j in range(2)]
        for c2 in range(NE2):
            for j in range(2):
                nc.tensor.matmul(
                    psy[j], g.catT[:, 2 * c2:2 * c2 + 2, trows],
                    pw_sb[:, c2, :, j * 512:(j + 1) * 512],
                    start=(c2 == 0), stop=(c2 == NE2 - 1), perf_mode=DR,
                )
        x2 = g.x2_tiles[ts]
        for j in range(2):
            jc = slice(j * 512, (j + 1) * 512)
            # scalar engine: x2 = psy/256  (fp8 scale compensation)
            nc.scalar.activation(out=x2[:, jc], in_=psy[j], func=AF.Copy,
                                 scale=PROJ_OUT_SCALE)
            if g.has_pb:
                nc.vector.tensor_add(out=x2[:, jc], in0=x2[:, jc],
                                     in1=g.pb_bc[:, jc])
            nc.vector.tensor_add(out=x2[:, jc], in0=x2[:, jc], in1=xres[:, jc])
        h2 = h2p.tile([P, E], bf16, name="h2")
        _emit_ln(g, x2, h2)
        for c in range(NE):
            tp = t2ps.tile([P, P], bf16, name="t2")
            nc.tensor.transpose(tp, h2[:, c * P:(c + 1) * P], g.ident)
            nc.vector.tensor_copy(out=g.h2T[c][:, trows], in_=tp)


def _emit_mlp(g, hidp, dwpp, outp, upps, dnps):
    nc = g.nc
    TQQ = 256  # token quarter
    for q in range(4):
        qcols = slice(q * TQQ, (q + 1) * TQQ)
        dn = [dnps.tile([P, E], f32, name=f"dn{j}") for j in range(2)]
        for f in range(NF):
            pu = upps.tile([P, TQQ], f32, name="pu")
            for c in range(NE):
                nc.tensor.matmul(
                    pu, g.uw_sb[c][:, f * P:(f + 1) * P], g.h2T[c][:, qcols],
                    start=(c == 0), stop=(c == NE - 1),
                )
            hid = hidp.tile([P, TQQ], bf16, name="hid")
            nc.scalar.activation(out=hid, in_=pu, func=AF.Relu,
                                 bias=g.ub_sb[:, f:f + 1])
            dwt = dwpp.tile([P, E], bf16, name="dwt")
            nc.sync.dma_start(out=dwt, in_=g.dw_d[f])
            for t2 in range(2):
                for j in range(2):
                    nc.tensor.matmul(
                        dn[t2][:, j * 512:(j + 1) * 512],
                        hid[:, t2 * P:(t2 + 1) * P],
                        dwt[:, j * 512:(j + 1) * 512],
                        start=(f == 0), stop=(f == NF - 1),
                    )
        for t2 in range(2):
            ti = q * 2 + t2
            ot = outp.tile([P, E], f32, name="ot")
            if g.has_db:
                nc.vector.tensor_add(out=ot, in0=dn[t2], in1=g.db_bc)
                nc.vector.tensor_add(out=ot, in0=ot, in1=g.x2_tiles[ti])
            else:
                nc.vector.tensor_add(out=ot, in0=dn[t2], in1=g.x2_tiles[ti])
            nc.sync.dma_start(out=g.out_d[ti * P:(ti + 1) * P, :], in_=ot)


def _build(flags, reps=1):
    has_qb, has_pb, has_db = flags
    nc = bacc.Bacc("TRN2", target_bir_lowering=False, debug=False, num_devices=8)

    g = _Ctx()
    g.nc = nc
    g.has_qb, g.has_pb, g.has_db = flags
    g.xkv_d = nc.dram_tensor("xkv", [T, E], f32, kind="ExternalInput")
    g.wq_d = nc.dram_tensor("wq", [NE2, NPAIR, P, 2, P], f8, kind="ExternalInput")
    g.wk_d = nc.dram_tensor("wk", [NE2, NPAIR, P, 2, P], f8, kind="ExternalInput")
    g.wv_d = nc.dram_tensor("wv", [NE2, P, 2, E], f8, kind="ExternalInput")
    g.vrow_d = nc.dram_tensor("vrow", [1, 2 * H * VW], f8, kind="ExternalInput")
    g.pw_d = nc.dram_tensor("pw", [NE2, P, 2, E], f8, kind="ExternalInput")
    g.uw_d = nc.dram_tensor("uw", [NE, P, F], bf16, kind="ExternalInput")
    g.ub_d = nc.dram_tensor("ub", [P, NF], f32, kind="ExternalInput")
    g.dw_d = nc.dram_tensor("dw", [NF, P, E], bf16, kind="ExternalInput")
    if has_qb:
        g.qb_d = nc.dram_tensor("qb", [P, NPAIR], f32, kind="ExternalInput")
        g.kb_d = nc.dram_tensor("kb", [P, NPAIR], f32, kind="ExternalInput")
        g.vbrow_d = nc.dram_tensor("vbrow", [1, E], bf16, kind="ExternalInput")
    if has_pb:
        g.pbrow_d = nc.dram_tensor("pbrow", [1, E], f32, kind="ExternalInput")
    if has_db:
        g.dbrow_d = nc.dram_tensor("dbrow", [1, E], f32, kind="ExternalInput")
    g.out_d = nc.dram_tensor("out", [TQ, E], f32, kind="ExternalOutput")

    with tile.TileContext(nc) as tc:
        with (
            tc.tile_pool(name="consts", bufs=1) as consts,
            tc.tile_pool(name="stat", bufs=4) as stat,
            tc.tile_pool(name="catp", bufs=1) as catp,
            tc.tile_pool(name="x2p", bufs=1) as x2p,
            tc.tile_pool(name="h2Tp", bufs=1) as h2Tp,
        ):
            g.consts, g.stat = consts, stat
            _emit_consts(g)
            for _rep in range(reps):
                _emit_all(g, tc, catp, x2p, h2Tp)

    nc.finalize()
    return nc


def _emit_all(g, tc, catp, x2p, h2Tp):
    g.catT = catp.tile([P, NPAIR, TQ], f8, name="catT")
    g.x2_tiles = [x2p.tile([P, E], f32, name=f"x2_{i}") for i in range(NTS)]
    g.h2T = [h2Tp.tile([P, TQ], bf16, name=f"h2T{c}") for c in range(NE)]

    g.pwp = tc.alloc_tile_pool(name="pwp", bufs=1)
    with (
        tc.tile_pool(name="hp", bufs=4) as hp,
        tc.tile_pool(name="hTp", bufs=1) as hTp,
        tc.tile_pool(name="vaug", bufs=1) as vap,
    ):
        g.hp = hp
        g.hT = hTp.tile([P, NE, T], f8, name="hT")
        with (
            tc.tile_pool(name="xk", bufs=3) as xkp,
            tc.tile_pool(name="tps", bufs=2, space="PSUM") as tps,
        ):
            _emit_ln1_transpose(g, xkp, tps)

        g.va = [vap.tile([P, 2, H * VW], f8, name=f"va{sp}")
                for sp in range(NSP)]
        with (
            tc.tile_pool(name="wvp", bufs=1) as wvp,
            tc.tile_pool(name="vps", bufs=4, space="PSUM") as vps,
        ):
            _emit_v(g, wvp, vps)

        with (
            tc.tile_pool(name="wqk", bufs=6) as wqkp,
            tc.tile_pool(name="qtp", bufs=2) as qtp,
            tc.tile_pool(name="ktp", bufs=2) as ktp,
            tc.tile_pool(name="ptp", bufs=4) as ptp,
            tc.tile_pool(name="smp", bufs=2) as smp,
            tc.tile_pool(name="qaps", bufs=2, space="PSUM") as qaps,
            tc.tile_pool(name="scps", bufs=2, space="PSUM") as scps,
        ):
            for p in range(NPAIR):
                qt = qtp.tile([P, TQ], f8, name="qt")
                kt = ktp.tile([P, T], f8, name="kt")
                _emit_qkt_pair(g, p, qt, kt, wqkp, qaps)
                _emit_attn_pair(g, p, qt, kt, ptp, smp, scps, qaps)
                if p == 0:
                    # prefetch proj weights on the idle SWDGE queue so the
                    # proj phase doesn't stall on them later
                    g.pw_sb = g.pwp.tile([P, NE2, 2, E], f8, name="pw")
                    for c2 in range(NE2):
                        g.nc.gpsimd.dma_start(out=g.pw_sb[:, c2],
                                              in_=g.pw_d[c2])

    with (
        tc.tile_pool(name="uwp", bufs=1) as uwp,
        tc.tile_pool(name="xq2", bufs=3) as xq2p,
        tc.tile_pool(name="h2p", bufs=3) as h2p,
    ):
        with (
            tc.tile_pool(name="pps", bufs=2, space="PSUM") as pps,
            tc.tile_pool(name="t2ps", bufs=2, space="PSUM") as t2ps,
        ):
            _emit_proj_ln2(g, uwp, xq2p, h2p, pps, t2ps)

        with (
            tc.tile_pool(name="hidp", bufs=6) as hidp,
            tc.tile_pool(name="dwpp", bufs=4) as dwpp,
            tc.tile_pool(name="outp", bufs=3) as outp,
            tc.tile_pool(name="upps", bufs=3, space="PSUM") as upps,
            tc.tile_pool(name="dnps", bufs=1, space="PSUM") as dnps,
        ):
            _emit_mlp(g, hidp, dwpp, outp, upps, dnps)
    g.pwp.release()


def _get_nc(flags, reps=1):
    key = (flags, reps)
    if key not in _BUILD_CACHE:
        _BUILD_CACHE[key] = _build(flags, reps)
    return _BUILD_CACHE[key]


def _prep(x, Wq, Wk, Wv, proj_w, proj_b, ln1_g, ln1_b, ln2_g, ln2_b,
          up_w, up_b, down_w, down_b):
    """Host-side shard + weight fold/cast/layout. Returns (flags, in_maps)."""
    bfl = ml_dtypes.bfloat16
    f8l = ml_dtypes.float8_e4m3fn
    x = np.ascontiguousarray(np.asarray(x, dtype=np.float32))
    Wq = np.asarray(Wq, np.float32)
    Wk = np.asarray(Wk, np.float32)
    Wv = np.asarray(Wv, np.float32)
    g1 = np.asarray(ln1_g, np.float32)
    b1 = np.asarray(ln1_b, np.float32)
    g2 = np.asarray(ln2_g, np.float32)
    b2 = np.asarray(ln2_b, np.float32)
    proj_w = np.asarray(proj_w, np.float32)
    up_w = np.asarray(up_w, np.float32)
    down_w = np.asarray(down_w, np.float32)

    # [H, E, D] -> [E, H*D]; fold LN1 gain + fp8 range scales into weights.
    # (1/sqrt(D) and the QS^2 compensation fold into the Exp activation.)
    wq_all = Wq.transpose(1, 0, 2).reshape(E, E) * QS
    wk_all = Wk.transpose(1, 0, 2).reshape(E, E) * QS
    wv_all = Wv.transpose(1, 0, 2).reshape(E, E) * VS
    qb_vec = b1 @ wq_all
    kb_vec = b1 @ wk_all
    vb_vec = b1 @ wv_all
    wq_f = g1[:, None] * wq_all
    wk_f = g1[:, None] * wk_all
    wv_f = g1[:, None] * wv_all

    def _qk_chunks(w):  # [E, E] -> [NE2, NPAIR, P, 2, P]
        return np.ascontiguousarray(
            w.reshape(NE2, 2, P, NPAIR, P).transpose(0, 3, 2, 1, 4).astype(f8l)
        )

    def _row_chunks(w):  # [E, N] -> [NE2, P, 2, N]
        n = w.shape[1]
        return np.ascontiguousarray(
            w.reshape(NE2, 2, P, n).transpose(0, 2, 1, 3).astype(f8l)
        )

    vrow = np.zeros((1, 2 * H * VW), np.float32)
    vrow.reshape(2, H, VW)[:, :, D] = 1.0

    uw_f = g2[:, None] * up_w
    ub_f = np.asarray(up_b, np.float32) + b2 @ up_w

    has_qb = bool(np.any(b1 != 0))
    has_pb = bool(np.any(np.asarray(proj_b) != 0))
    has_db = bool(np.any(np.asarray(down_b) != 0))
    flags = (has_qb, has_pb, has_db)

    shared = {
        "wq": _qk_chunks(wq_f),
        "wk": _qk_chunks(wk_f),
        "wv": _row_chunks(wv_f),
        "vrow": vrow.astype(f8l),
        "pw": _row_chunks(proj_w * PS),
        "uw": np.ascontiguousarray(uw_f.reshape(NE, P, F).astype(bfl)),
        "ub": np.ascontiguousarray(ub_f.reshape(NF, P).T.astype(np.float32)),
        "dw": np.ascontiguousarray(down_w.reshape(NF, P, E).astype(bfl)),
    }
    if has_qb:
        shared["qb"] = np.ascontiguousarray(
            qb_vec.reshape(NPAIR, P).T.astype(np.float32))
        shared["kb"] = np.ascontiguousarray(
            kb_vec.reshape(NPAIR, P).T.astype(np.float32))
        shared["vbrow"] = vb_vec.reshape(1, E).astype(bfl)
    if has_pb:
        shared["pbrow"] = np.asarray(proj_b, np.float32).reshape(1, E)
    if has_db:
        shared["dbrow"] = np.asarray(down_b, np.float32).reshape(1, E)

    in_maps = []
    for c in range(8):
        b, half = c // 2, c % 2
        xb = x[b]
        if half == 1:
            xb = np.concatenate([xb[TQ:], xb[:TQ]], axis=0)
        in_maps.append({"xkv": np.ascontiguousarray(xb), **shared})
    return flags, in_maps


def kernel(**inputs) -> np.ndarray:
    flags, in_maps = _prep(**inputs)
    nc = _get_nc(flags)
    res = run_bass_kernel_spmd(nc, in_maps, core_ids=list(range(8)))
    out = np.empty((B, T, E), np.float32)
    for c in range(8):
        b, half = c // 2, c % 2
        out[b, half * TQ:(half + 1) * TQ, :] = res.results[c]["out"]
    return out


# revision 11
# speedup vs baseline: 1.0533x; 1.0533x over previous
"""Trainium2 Bass kernel for a dense transformer block (nn_Block_120259084502).

Contract: kernel(**inputs) takes the FULL unsharded inputs (numpy, fp32) and
returns the FULL output [4, 2048, 1024] fp32. Internally shards across 8
NeuronCores: core c handles batch c//2, query-token half c%2. Each core
receives its batch's full 2048 tokens (rolled so its own 1024 query tokens
come first) and computes K/V for all of them locally, so no collectives are
needed (attention context = full batch; softmax is order-invariant so the
roll is harmless).

Attention path (QKV, scores, exp*V, proj) runs in fp8 e4m3 with DoubleRow
perf mode (2x matmul rate) where the contraction allows; the MLP stays bf16
(fp8 there would exceed the error budget). PSUM accumulation is fp32
everywhere; LayerNorm statistics and residual adds stay fp32. Host-side
scale folding keeps every fp8 tensor in e4m3's normal range:
  wq,wk scaled x8 (the 1/sqrt(D) and 1/64 compensations fold into the Exp
  activation's scale), wv and proj_w scaled x16 (compensated by a 1/256
  scale on the proj PSUM copy-out).
"""

import numpy as np
import ml_dtypes

import concourse.bacc as bacc
import concourse.tile as tile
from concourse import mybir
from concourse.bass_utils import run_bass_kernel_spmd
from concourse.masks import make_identity

bf16 = mybir.dt.bfloat16
f8 = mybir.dt.float8e4
f32 = mybir.dt.float32
AF = mybir.ActivationFunctionType
ALU = mybir.AluOpType
DR = mybir.MatmulPerfMode.DoubleRow

P = 128
B, T, E, H, D = 4, 2048, 1024, 16, 64
F = 4 * E                    # 4096 MLP hidden
TQ = T // 2                  # 1024 own query tokens per core
NE = E // P                  # 8 e-chunks
NE2 = NE // 2                # 4 e-chunk pairs (DoubleRow)
NPAIR = H // 2               # 8 head pairs
NST = T // P                 # 16 context-token tiles
NSP = NST // 2               # 8 context-token tile pairs
NTS = TQ // P                # 8 own-token tiles
NF = F // P                  # 32 f-chunks
VW = D + 2                   # per-head V width incl. ones column + pad
                             # (DoubleRow ldweights needs an even M)
LN_EPS = 1e-5
QS = 8.0                     # host scale on wq,wk
VS = 16.0                    # host scale on wv
PS = 16.0                    # host scale on proj_w
EXP_SCALE = (D ** -0.5) / (QS * QS)
EXP_BIAS = -3.5              # keeps exp outputs in e4m3 range (softmax-invariant)
PROJ_OUT_SCALE = 1.0 / (VS * PS)

_BUILD_CACHE = {}


class _Ctx:
    """Shared build state passed between phase emitters."""
    pass


def _emit_ln(g, xt, out_lp):
    nc = g.nc
    st = g.stat.tile([P, 2, nc.vector.BN_STATS_DIM], f32, name="bnst")
    xv = xt.rearrange("p (s g) -> p s g", s=2)
    nc.vector.bn_stats(out=st[:, 0, :], in_=xv[:, 0, :])
    nc.vector.bn_stats(out=st[:, 1, :], in_=xv[:, 1, :])
    mv = g.stat.tile([P, nc.vector.BN_AGGR_DIM], f32, name="bnmv")
    nc.vector.bn_aggr(out=mv, in_=st)
    rstd = g.stat.tile([P, 1], f32, name="bnrs")
    nc.scalar.activation(out=rstd, in_=mv[:, 1:2], func=AF.Sqrt, bias=g.eps_t)
    nc.vector.reciprocal(out=rstd, in_=rstd)
    nc.vector.tensor_scalar(
        out=out_lp, in0=xt, scalar1=mv[:, 0:1], scalar2=rstd,
        op0=ALU.subtract, op1=ALU.mult,
    )


def _emit_consts(g):
    nc, consts = g.nc, g.consts
    g.ident = consts.tile([P, P], bf16, name="ident")
    make_identity(nc, g.ident)
    g.eps_t = consts.tile([P, 1], f32, name="eps")
    nc.vector.memset(g.eps_t, LN_EPS)
    g.expb_t = consts.tile([P, 1], f32, name="expb")
    nc.vector.memset(g.expb_t, EXP_BIAS)
    g.ub_sb = consts.tile([P, NF], f32, name="ubsb")
    nc.sync.dma_start(out=g.ub_sb, in_=g.ub_d[:, :])
    if g.has_qb:
        g.qb_sb = consts.tile([P, NPAIR], f32, name="qbsb")
        nc.sync.dma_start(out=g.qb_sb, in_=g.qb_d[:, :])
        g.kb_sb = consts.tile([P, NPAIR], f32, name="kbsb")
        nc.sync.dma_start(out=g.kb_sb, in_=g.kb_d[:, :])
        g.vb_bc = consts.tile([P, E], bf16, name="vbbc")
        nc.gpsimd.dma_start(
            out=g.vb_bc, in_=g.vbrow_d.ap()[0:1, :].partition_broadcast(P)[:, 0, :]
        )
    if g.has_pb:
        g.pb_bc = consts.tile([P, E], f32, name="pbbc")
        nc.gpsimd.dma_start(
            out=g.pb_bc, in_=g.pbrow_d.ap()[0:1, :].partition_broadcast(P)[:, 0, :]
        )
    if g.has_db:
        g.db_bc = consts.tile([P, E], f32, name="dbbc")
        nc.gpsimd.dma_start(
            out=g.db_bc, in_=g.dbrow_d.ap()[0:1, :].partition_broadcast(P)[:, 0, :]
        )


def _emit_ln1_transpose(g, xkp, tps):
    """Load x, LN1, PE-transpose h into e-major hT (fp8)."""
    nc = g.nc
    for i in range(NST):
        xt = xkp.tile([P, E], f32, name="xk")
        nc.sync.dma_start(out=xt, in_=g.xkv_d[i * P:(i + 1) * P, :])
        ht = g.hp.tile([P, E], bf16, name="h")
        _emit_ln(g, xt, ht)
        for c in range(NE):
            # fp8 transposes are illegal; transpose bf16, cast on copy-out
            tp = tps.tile([P, P], bf16, name="tp")
            nc.tensor.transpose(tp, ht[:, c * P:(c + 1) * P], g.ident)
            nc.vector.tensor_copy(out=g.hT[:, c, i * P:(i + 1) * P], in_=tp)


def _emit_v(g, wvp, vps):
    """V (x16) in natural [s, d] layout, paired s-tiles, ones col per head."""
    nc = g.nc
    wv_sb = wvp.tile([P, NE2, 2, E], f8, name="wv")
    for c2 in range(NE2):
        nc.sync.dma_start(out=wv_sb[:, c2], in_=g.wv_d[c2])
    for sp in range(NSP):
        nc.gpsimd.dma_start(
            out=g.va[sp],
            in_=g.vrow_d.ap()[0:1, :].partition_broadcast(P)[:, 0, :],
        )
        vav = g.va[sp].rearrange("p i (h c) -> p i h c", c=VW)
        for i in range(2):
            s = 2 * sp + i
            pv = [vps.tile([P, 512], f32, name=f"pv{j}") for j in range(2)]
            for c2 in range(NE2):
                for j in range(2):
                    nc.tensor.matmul(
                        pv[j], g.hT[:, 2 * c2:2 * c2 + 2, s * P:(s + 1) * P],
                        wv_sb[:, c2, :, j * 512:(j + 1) * 512],
                        start=(c2 == 0), stop=(c2 == NE2 - 1), perf_mode=DR,
                    )
            for j in range(2):
                dst = vav[:, i, j * 8:(j + 1) * 8, 0:D]
                src = pv[j].rearrange("p (h d) -> p h d", d=D)
                if g.has_qb:
                    vb_view = g.vb_bc.rearrange("p (h d) -> p h d", d=D)[
                        :, j * 8:(j + 1) * 8, :
                    ]
                    nc.vector.tensor_add(out=dst, in0=src, in1=vb_view)
                else:
                    nc.vector.tensor_copy(out=dst, in_=src)


def _emit_qkt_pair(g, p, qt, kt, wqkp, qkps):
    """Q^T and K^T for head pair p: [128 (2 heads x 64d), tokens], fp8."""
    nc = g.nc
    psq = [qkps.tile([P, 512], f32, name=f"ps{j}") for j in range(2)]
    for c2 in range(NE2):
        wsl = wqkp.tile([P, 2, P], f8, name="wsl")
        nc.sync.dma_start(out=wsl, in_=g.wq_d[c2, p])
        for j in range(2):
            nc.tensor.matmul(
                psq[j], wsl, g.hT[:, 2 * c2:2 * c2 + 2, j * 512:(j + 1) * 512],
                start=(c2 == 0), stop=(c2 == NE2 - 1), perf_mode=DR,
            )
    for j in range(2):
        dst = qt[:, j * 512:(j + 1) * 512]
        if g.has_qb:
            nc.vector.tensor_scalar(
                out=dst, in0=psq[j], scalar1=g.qb_sb[:, p:p + 1], op0=ALU.add
            )
        else:
            nc.vector.tensor_copy(out=dst, in_=psq[j])
    for sh in range(2):
        psk = [qkps.tile([P, 512], f32, name=f"ps{j}") for j in range(2)]
        for c2 in range(NE2):
            wsl = wqkp.tile([P, 2, P], f8, name="wsl")
            nc.sync.dma_start(out=wsl, in_=g.wk_d[c2, p])
            for j in range(2):
                s0 = (sh * 2 + j) * 512
                nc.tensor.matmul(
                    psk[j], wsl, g.hT[:, 2 * c2:2 * c2 + 2, s0:s0 + 512],
                    start=(c2 == 0), stop=(c2 == NE2 - 1), perf_mode=DR,
                )
        for j in range(2):
            s0 = (sh * 2 + j) * 512
            dst = kt[:, s0:s0 + 512]
            if g.has_qb:
                nc.vector.tensor_scalar(
                    out=dst, in0=psk[j], scalar1=g.kb_sb[:, p:p + 1], op0=ALU.add
                )
            else:
                nc.vector.tensor_copy(out=dst, in_=psk[j])


def _emit_attn_pair(g, p, qt, kt, ptp, smp, scps, atps):
    """Scores (transposed), exp (fp8), attn^T via DoubleRow over s-pairs,
    softmax denom from ones column, normalize -> catT."""
    nc = g.nc
    for th in range(2):
        tcols = slice(th * 512, (th + 1) * 512)
        at0 = atps.tile([VW, 512], f32, name="ps0")
        at1 = atps.tile([VW, 512], f32, name="ps1")
        for sp in range(NSP):
            pt0 = ptp.tile([P, 2, 512], f8, name="pt0")
            pt1 = ptp.tile([P, 2, 512], f8, name="pt1")
            for i in range(2):
                s = 2 * sp + i
                scols = slice(s * P, (s + 1) * P)
                sc0 = scps.tile([P, 512], f32, name="sc0")
                sc1 = scps.tile([P, 512], f32, name="sc1")
                # S^T[s,t] = (K^T slice).T @ Q^T slice; the two heads live on
                # row-groups 0-63 / 64-127 so the matmuls pack concurrently.
                nc.tensor.matmul(sc0, kt[0:D, scols], qt[0:D, tcols],
                                 start=True, stop=True)
                nc.tensor.matmul(sc1, kt[D:2 * D, scols], qt[D:2 * D, tcols],
                                 start=True, stop=True)
                nc.scalar.activation(out=pt0[:, i, :], in_=sc0, func=AF.Exp,
                                     scale=EXP_SCALE, bias=g.expb_t)
                nc.scalar.activation(out=pt1[:, i, :], in_=sc1, func=AF.Exp,
                                     scale=EXP_SCALE, bias=g.expb_t)
            nc.tensor.matmul(
                at0, g.va[sp][:, :, (2 * p) * VW:(2 * p) * VW + VW], pt0,
                start=(sp == 0), stop=(sp == NSP - 1), perf_mode=DR,
            )
            nc.tensor.matmul(
                at1, g.va[sp][:, :, (2 * p + 1) * VW:(2 * p + 1) * VW + VW], pt1,
                start=(sp == 0), stop=(sp == NSP - 1), perf_mode=DR,
            )
        se0 = smp.tile([1, 512], f32, name="se0")
        se1 = smp.tile([1, 512], f32, name="se1")
        nc.vector.reciprocal(out=se0, in_=at0[D:D + 1, :])
        nc.vector.reciprocal(out=se1, in_=at1[D:D + 1, :])
        rb0 = smp.tile([D, 512], f32, name="rb0")
        rb1 = smp.tile([D, 512], f32, name="rb1")
        nc.gpsimd.partition_broadcast(rb0, se0)
        nc.gpsimd.partition_broadcast(rb1, se1)
        nc.vector.tensor_mul(out=g.catT[0:D, p, tcols], in0=at0[0:D, :], in1=rb0)
        nc.vector.tensor_mul(out=g.catT[D:2 * D, p, tcols], in0=at1[0:D, :],
                             in1=rb1)


def _emit_proj_ln2(g, uwp, xq2p, h2p, pps, t2ps):
    nc = g.nc
    pw_sb = g.pw_sb
    g.uw_sb = []
    for c in range(NE):  # prefetch MLP up-weights while proj runs
        w = uwp.tile([P, F], bf16, name=f"uw{c}")
        nc.sync.dma_start(out=w, in_=g.uw_d[c])
        g.uw_sb.append(w)
    for ts in range(NTS):
        trows = slice(ts * P, (ts + 1) * P)
        xres = xq2p.tile([P, E], f32, name="xres")
        nc.sync.dma_start(out=xres, in_=g.xkv_d[ts * P:(ts + 1) * P, :])
        psy = [pps.tile([P, 512], f32, name=f"py{j}") for j in range(2)]
        for c2 in range(NE2):
            for j in range(2):
                nc.tensor.matmul(
                    psy[j], g.catT[:, 2 * c2:2 * c2 + 2, trows],
                    pw_sb[:, c2, :, j * 512:(j + 1) * 512],
                    start=(c2 == 0), stop=(c2 == NE2 - 1), perf_mode=DR,
                )
        x2 = g.x2_tiles[ts]
        for j in range(2):
            jc = slice(j * 512, (j + 1) * 512)
            # scalar engine: x2 = psy/256  (fp8 scale compensation)
            nc.scalar.activation(out=x2[:, jc], in_=psy[j], func=AF.Copy,
                                 scale=PROJ_OUT_SCALE)
            if g.has_pb:
                nc.vector.tensor_add(out=x2[:, jc], in0=x2[:, jc],
                                     in1=g.pb_bc[:, jc])
            nc.vector.tensor_add(out=x2[:, jc], in0=x2[:, jc], in1=xres[:, jc])
        h2 = h2p.tile([P, E], bf16, name="h2")
        _emit_ln(g, x2, h2)
        for c in range(NE):
            tp = t2ps.tile([P, P], bf16, name="t2")
            nc.tensor.transpose(tp, h2[:, c * P:(c + 1) * P], g.ident)
            nc.vector.tensor_copy(out=g.h2T[c][:, trows], in_=tp)


def _emit_mlp(g, hidp, dwpp, outp, upps, dnps):
    nc = g.nc
    TQQ = 256  # token quarter
    for q in range(4):
        qcols = slice(q * TQQ, (q + 1) * TQQ)
        dn = [dnps.tile([P, E], f32, name=f"dn{j}") for j in range(2)]
        for f in range(NF):
            pu = upps.tile([P, TQQ], f32, name="pu")
            for c in range(NE):
                nc.tensor.matmul(
                    pu, g.uw_sb[c][:, f * P:(f + 1) * P], g.h2T[c][:, qcols],
                    start=(c == 0), stop=(c == NE - 1),
                )
            hid = hidp.tile([P, TQQ], bf16, name="hid")
            nc.scalar.activation(out=hid, in_=pu, func=AF.Relu,
                                 bias=g.ub_sb[:, f:f + 1])
            dwt = dwpp.tile([P, E], bf16, name="dwt")
            nc.sync.dma_start(out=dwt, in_=g.dw_d[f])
            for t2 in range(2):
                for j in range(2):
                    nc.tensor.matmul(
                        dn[t2][:, j * 512:(j + 1) * 512],
                        hid[:, t2 * P:(t2 + 1) * P],
                        dwt[:, j * 512:(j + 1) * 512],
                        start=(f == 0), stop=(f == NF - 1),
                    )
        for t2 in range(2):
            ti = q * 2 + t2
            ot = outp.tile([P, E], f32, name="ot")
            if g.has_db:
                nc.vector.tensor_add(out=ot, in0=dn[t2], in1=g.db_bc)
                nc.vector.tensor_add(out=ot, in0=ot, in1=g.x2_tiles[ti])
            else:
                nc.vector.tensor_add(out=ot, in0=dn[t2], in1=g.x2_tiles[ti])
            nc.sync.dma_start(out=g.out_d[ti * P:(ti + 1) * P, :], in_=ot)


def _build(flags, reps=1):
    has_qb, has_pb, has_db = flags
    nc = bacc.Bacc("TRN2", target_bir_lowering=False, debug=False, num_devices=8)

    g = _Ctx()
    g.nc = nc
    g.has_qb, g.has_pb, g.has_db = flags
    g.xkv_d = nc.dram_tensor("xkv", [T, E], f32, kind="ExternalInput")
    g.wq_d = nc.dram_tensor("wq", [NE2, NPAIR, P, 2, P], f8, kind="ExternalInput")
    g.wk_d = nc.dram_tensor("wk", [NE2, NPAIR, P, 2, P], f8, kind="ExternalInput")
    g.wv_d = nc.dram_tensor("wv", [NE2, P, 2, E], f8, kind="ExternalInput")
    g.vrow_d = nc.dram_tensor("vrow", [1, 2 * H * VW], f8, kind="ExternalInput")
    g.pw_d = nc.dram_tensor("pw", [NE2, P, 2, E], f8, kind="ExternalInput")
    g.uw_d = nc.dram_tensor("uw", [NE, P, F], bf16, kind="ExternalInput")
    g.ub_d = nc.dram_tensor("ub", [P, NF], f32, kind="ExternalInput")
    g.dw_d = nc.dram_tensor("dw", [NF, P, E], bf16, kind="ExternalInput")
    if has_qb:
        g.qb_d = nc.dram_tensor("qb", [P, NPAIR], f32, kind="ExternalInput")
        g.kb_d = nc.dram_tensor("kb", [P, NPAIR], f32, kind="ExternalInput")
        g.vbrow_d = nc.dram_tensor("vbrow", [1, E], bf16, kind="ExternalInput")
    if has_pb:
        g.pbrow_d = nc.dram_tensor("pbrow", [1, E], f32, kind="ExternalInput")
    if has_db:
        g.dbrow_d = nc.dram_tensor("dbrow", [1, E], f32, kind="ExternalInput")
    g.out_d = nc.dram_tensor("out", [TQ, E], f32, kind="ExternalOutput")

    with tile.TileContext(nc) as tc:
        with (
            tc.tile_pool(name="consts", bufs=1) as consts,
            tc.tile_pool(name="stat", bufs=4) as stat,
            tc.tile_pool(name="catp", bufs=1) as catp,
            tc.tile_pool(name="x2p", bufs=1) as x2p,
            tc.tile_pool(name="h2Tp", bufs=1) as h2Tp,
        ):
            g.consts, g.stat = consts, stat
            _emit_consts(g)
            for _rep in range(reps):
                _emit_all(g, tc, catp, x2p, h2Tp)

    nc.finalize()
    return nc


def _emit_all(g, tc, catp, x2p, h2Tp):
    g.catT = catp.tile([P, NPAIR, TQ], f8, name="catT")
    g.x2_tiles = [x2p.tile([P, E], f32, name=f"x2_{i}") for i in range(NTS)]
    g.h2T = [h2Tp.tile([P, TQ], bf16, name=f"h2T{c}") for c in range(NE)]

    g.pwp = tc.alloc_tile_pool(name="pwp", bufs=1)
    with (
        tc.tile_pool(name="hp", bufs=4) as hp,
        tc.tile_pool(name="hTp", bufs=1) as hTp,
        tc.tile_pool(name="vaug", bufs=1) as vap,
    ):
        g.hp = hp
        g.hT = hTp.tile([P, NE, T], f8, name="hT")
        with (
            tc.tile_pool(name="xk", bufs=3) as xkp,
            tc.tile_pool(name="tps", bufs=2, space="PSUM") as tps,
        ):
            _emit_ln1_transpose(g, xkp, tps)

        g.va = [vap.tile([P, 2, H * VW], f8, name=f"va{sp}")
                for sp in range(NSP)]
        with (
            tc.tile_pool(name="wvp", bufs=1) as wvp,
            tc.tile_pool(name="vps", bufs=4, space="PSUM") as vps,
        ):
            _emit_v(g, wvp, vps)

        with (
            tc.tile_pool(name="wqk", bufs=6) as wqkp,
            tc.tile_pool(name="qtp", bufs=2) as qtp,
            tc.tile_pool(name="ktp", bufs=2) as ktp,
            tc.tile_pool(name="ptp", bufs=4) as ptp,
            tc.tile_pool(name="smp", bufs=2) as smp,
            tc.tile_pool(name="qaps", bufs=2, space="PSUM") as qaps,
            tc.tile_pool(name="scps", bufs=2, space="PSUM") as scps,
        ):
            for p in range(NPAIR):
                qt = qtp.tile([P, TQ], f8, name="qt")
                kt = ktp.tile([P, T], f8, name="kt")
                _emit_qkt_pair(g, p, qt, kt, wqkp, qaps)
                _emit_attn_pair(g, p, qt, kt, ptp, smp, scps, qaps)
                if p == 0:
                    # prefetch proj weights on the idle SWDGE queue so the
                    # proj phase doesn't stall on them later
                    g.pw_sb = g.pwp.tile([P, NE2, 2, E], f8, name="pw")
                    for c2 in range(NE2):
                        g.nc.gpsimd.dma_start(out=g.pw_sb[:, c2],
                                              in_=g.pw_d[c2])

    with (
        tc.tile_pool(name="uwp", bufs=1) as uwp,
        tc.tile_pool(name="xq2", bufs=3) as xq2p,
        tc.tile_pool(name="h2p", bufs=3) as h2p,
    ):
        with (
            tc.tile_pool(name="pps", bufs=2, space="PSUM") as pps,
            tc.tile_pool(name="t2ps", bufs=2, space="PSUM") as t2ps,
        ):
            _emit_proj_ln2(g, uwp, xq2p, h2p, pps, t2ps)

        with (
            tc.tile_pool(name="hidp", bufs=6) as hidp,
            tc.tile_pool(name="dwpp", bufs=4) as dwpp,
            tc.tile_pool(name="outp", bufs=3) as outp,
            tc.tile_pool(name="upps", bufs=3, space="PSUM") as upps,
            tc.tile_pool(name="dnps", bufs=1, space="PSUM") as dnps,
        ):
            _emit_mlp(g, hidp, dwpp, outp, upps, dnps)
    g.pwp.release()


def _get_nc(flags, reps=1):
    key = (flags, reps)
    if key not in _BUILD_CACHE:
        _BUILD_CACHE[key] = _build(flags, reps)
    return _BUILD_CACHE[key]


def _prep(x, Wq, Wk, Wv, proj_w, proj_b, ln1_g, ln1_b, ln2_g, ln2_b,
          up_w, up_b, down_w, down_b):
    """Host-side shard + weight fold/cast/layout. Returns (flags, in_maps)."""
    bfl = ml_dtypes.bfloat16
    f8l = ml_dtypes.float8_e4m3fn
    x = np.ascontiguousarray(np.asarray(x, dtype=np.float32))
    Wq = np.asarray(Wq, np.float32)
    Wk = np.asarray(Wk, np.float32)
    Wv = np.asarray(Wv, np.float32)
    g1 = np.asarray(ln1_g, np.float32)
    b1 = np.asarray(ln1_b, np.float32)
    g2 = np.asarray(ln2_g, np.float32)
    b2 = np.asarray(ln2_b, np.float32)
    proj_w = np.asarray(proj_w, np.float32)
    up_w = np.asarray(up_w, np.float32)
    down_w = np.asarray(down_w, np.float32)

    # [H, E, D] -> [E, H*D]; fold LN1 gain + fp8 range scales into weights.
    # (1/sqrt(D) and the QS^2 compensation fold into the Exp activation.)
    wq_all = Wq.transpose(1, 0, 2).reshape(E, E) * QS
    wk_all = Wk.transpose(1, 0, 2).reshape(E, E) * QS
    wv_all = Wv.transpose(1, 0, 2).reshape(E, E) * VS
    qb_vec = b1 @ wq_all
    kb_vec = b1 @ wk_all
    vb_vec = b1 @ wv_all
    wq_f = g1[:, None] * wq_all
    wk_f = g1[:, None] * wk_all
    wv_f = g1[:, None] * wv_all

    def _qk_chunks(w):  # [E, E] -> [NE2, NPAIR, P, 2, P]
        return np.ascontiguousarray(
            w.reshape(NE2, 2, P, NPAIR, P).transpose(0, 3, 2, 1, 4).astype(f8l)
        )

    def _row_chunks(w):  # [E, N] -> [NE2, P, 2, N]
        n = w.shape[1]
        return np.ascontiguousarray(
            w.reshape(NE2, 2, P, n).transpose(0, 2, 1, 3).astype(f8l)
        )

    vrow = np.zeros((1, 2 * H * VW), np.float32)
    vrow.reshape(2, H, VW)[:, :, D] = 1.0

    uw_f = g2[:, None] * up_w
    ub_f = np.asarray(up_b, np.float32) + b2 @ up_w

    has_qb = bool(np.any(b1 != 0))
    has_pb = bool(np.any(np.asarray(proj_b) != 0))
    has_db = bool(np.any(np.asarray(down_b) != 0))
    flags = (has_qb, has_pb, has_db)

    shared = {
        "wq": _qk_chunks(wq_f),
        "wk": _qk_chunks(wk_f),
        "wv": _row_chunks(wv_f),
        "vrow": vrow.astype(f8l),
        "pw": _row_chunks(proj_w * PS),
        "uw": np.ascontiguousarray(uw_f.reshape(NE, P, F).astype(bfl)),
        "ub": np.ascontiguousarray(ub_f.reshape(NF, P).T.astype(np.float32)),
        "dw": np.ascontiguousarray(down_w.reshape(NF, P, E).astype(bfl)),
    }
    if has_qb:
        shared["qb"] = np.ascontiguousarray(
            qb_vec.reshape(NPAIR, P).T.astype(np.float32))
        shared["kb"] = np.ascontiguousarray(
            kb_vec.reshape(NPAIR, P).T.astype(np.float32))
        shared["vbrow"] = vb_vec.reshape(1, E).astype(bfl)
    if has_pb:
        shared["pbrow"] = np.asarray(proj_b, np.float32).reshape(1, E)
    if has_db:
        shared["dbrow"] = np.asarray(down_b, np.float32).reshape(1, E)

    in_maps = []
    for c in range(8):
        b, half = c // 2, c % 2
        xb = x[b]
        if half == 1:
            xb = np.concatenate([xb[TQ:], xb[:TQ]], axis=0)
        in_maps.append({"xkv": np.ascontiguousarray(xb), **shared})
    return flags, in_maps


def kernel(**inputs) -> np.ndarray:
    flags, in_maps = _prep(**inputs)
    nc = _get_nc(flags)
    res = run_bass_kernel_spmd(nc, in_maps, core_ids=list(range(8)))
    out = np.empty((B, T, E), np.float32)
    for c in range(8):
        b, half = c // 2, c % 2
        out[b, half * TQ:(half + 1) * TQ, :] = res.results[c]["out"]
    return out


# revision 21
# speedup vs baseline: 1.2785x; 1.2138x over previous
"""Trainium2 Bass kernel for a dense transformer block (nn_Block_120259084502).

Contract: kernel(**inputs) takes the FULL unsharded inputs (numpy, fp32) and
returns the FULL output [4, 2048, 1024] fp32. Internally shards across 8
NeuronCores: core c handles batch c//2, query-token half c%2. Each core
receives its batch's full 2048 tokens (rolled so its own 1024 query tokens
come first) and computes K/V for all of them locally, so no collectives are
needed (attention context = full batch; softmax is order-invariant so the
roll is harmless).

Attention path (QKV, scores, exp*V, proj) runs in fp8 e4m3, with DoubleRow
perf mode (2x matmul rate) wherever the contraction dim allows; the MLP
stays bf16 (fp8 there would exceed the error budget). PSUM accumulation is
fp32 everywhere; LayerNorm statistics stay fp32 (x itself is cast to bf16 on
the host — its 0.4% rounding is far below the fp8 path's quantization).

Engines execute their queues in order, and the softmax exp (scalar engine)
is the attention bottleneck, so emission order is arranged to keep the
tensor engine fed while the scalar engine churns through exps:
  - V projection matmuls are interleaved into the LN1/transpose loop,
  - per-pair Q/K projections are interleaved with attention query-half 0,
  - the MLP up-projections for query-half 0 (N=512, hid kept in SBUF bf16)
    are interleaved with attention query-half 1,
  - the down-projections then run with dw streamed once per f-chunk for two
    token quarters at a time (8 PSUM banks).

Host-side scale folding keeps every fp8 tensor in e4m3's normal range:
wq,wk x8 (1/sqrt(D) and the 1/64 compensation fold into the Exp
activation's scale), wv and proj_w x16 (compensated by a 1/256 scale on the
proj PSUM copy-out).
"""

import numpy as np
import ml_dtypes

import concourse.bacc as bacc
import concourse.tile as tile
from concourse import mybir
from concourse.bass_utils import run_bass_kernel_spmd
from concourse.masks import make_identity

bf16 = mybir.dt.bfloat16
f8 = mybir.dt.float8e4
f32 = mybir.dt.float32
AF = mybir.ActivationFunctionType
ALU = mybir.AluOpType
DR = mybir.MatmulPerfMode.DoubleRow

P = 128
B, T, E, H, D = 4, 2048, 1024, 16, 64
F = 4 * E                    # 4096 MLP hidden
TQ = T // 2                  # 1024 own query tokens per core
NE = E // P                  # 8 e-chunks
NE2 = NE // 2                # 4 e-chunk pairs (DoubleRow)
NPAIR = H // 2               # 8 head pairs
NST = T // P                 # 16 context-token tiles
NSP = NST // 2               # 8 context-token tile pairs
NTS = TQ // P                # 8 own-token tiles
NF = F // P                  # 32 f-chunks
VW = D + 2                   # per-head V width incl. ones column + pad
                             # (DoubleRow ldweights needs an even M)
LN_EPS = 1e-5
QS = 8.0                     # host scale on wq,wk
VS = 16.0                    # host scale on wv
PS = 16.0                    # host scale on proj_w
EXP_SCALE = (D ** -0.5) / (QS * QS)
EXP_BIAS = -3.5              # keeps exp outputs in e4m3 range (softmax-invariant)
PROJ_OUT_SCALE = 1.0 / (VS * PS)

_BUILD_CACHE = {}


class _Ctx:
    """Shared build state passed between phase emitters."""
    pass


def _emit_ln(g, xt, out_lp):
    nc = g.nc
    st = g.stat.tile([P, 2, nc.vector.BN_STATS_DIM], f32, name="bnst")
    xv = xt.rearrange("p (s g) -> p s g", s=2)
    nc.vector.bn_stats(out=st[:, 0, :], in_=xv[:, 0, :])
    nc.vector.bn_stats(out=st[:, 1, :], in_=xv[:, 1, :])
    mv = g.stat.tile([P, nc.vector.BN_AGGR_DIM], f32, name="bnmv")
    nc.vector.bn_aggr(out=mv, in_=st)
    rstd = g.stat.tile([P, 1], f32, name="bnrs")
    nc.scalar.activation(out=rstd, in_=mv[:, 1:2], func=AF.Sqrt, bias=g.eps_t)
    nc.vector.reciprocal(out=rstd, in_=rstd)
    nc.vector.tensor_scalar(
        out=out_lp, in0=xt, scalar1=mv[:, 0:1], scalar2=rstd,
        op0=ALU.subtract, op1=ALU.mult,
    )


def _emit_consts(g):
    nc, consts = g.nc, g.consts
    g.ident = consts.tile([P, P], bf16, name="ident")
    make_identity(nc, g.ident)
    g.eps_t = consts.tile([P, 1], f32, name="eps")
    nc.vector.memset(g.eps_t, LN_EPS)
    g.expb_t = consts.tile([P, 1], f32, name="expb")
    nc.vector.memset(g.expb_t, EXP_BIAS)
    g.ub_sb = consts.tile([P, NF], f32, name="ubsb")
    nc.sync.dma_start(out=g.ub_sb, in_=g.ub_d[:, :])
    if g.has_qb:
        g.qb_sb = consts.tile([P, NPAIR], f32, name="qbsb")
        nc.sync.dma_start(out=g.qb_sb, in_=g.qb_d[:, :])
        g.kb_sb = consts.tile([P, NPAIR], f32, name="kbsb")
        nc.sync.dma_start(out=g.kb_sb, in_=g.kb_d[:, :])
        g.vb_bc = consts.tile([P, E], bf16, name="vbbc")
        nc.gpsimd.dma_start(
            out=g.vb_bc, in_=g.vbrow_d.ap()[0:1, :].partition_broadcast(P)[:, 0, :]
        )
    if g.has_pb:
        g.pb_bc = consts.tile([P, E], f32, name="pbbc")
        nc.gpsimd.dma_start(
            out=g.pb_bc, in_=g.pbrow_d.ap()[0:1, :].partition_broadcast(P)[:, 0, :]
        )
    if g.has_db:
        g.db_bc = consts.tile([P, E], f32, name="dbbc")
        nc.gpsimd.dma_start(
            out=g.db_bc, in_=g.dbrow_d.ap()[0:1, :].partition_broadcast(P)[:, 0, :]
        )


def _emit_v_sp(g, sp, vps):
    """V (x16) for one s-tile pair, natural [s, d] layout, ones col/head."""
    nc = g.nc
    nc.gpsimd.dma_start(
        out=g.va[sp],
        in_=g.vrow_d.ap()[0:1, :].partition_broadcast(P)[:, 0, :],
    )
    vav = g.va[sp].rearrange("p i (h c) -> p i h c", c=VW)
    for i in range(2):
        s = 2 * sp + i
        pv = [vps.tile([P, 512], f32, name=f"pv{j}") for j in range(2)]
        for c2 in range(NE2):
            for j in range(2):
                nc.tensor.matmul(
                    pv[j], g.hT[:, 2 * c2:2 * c2 + 2, s * P:(s + 1) * P],
                    g.wv_sb[:, c2, :, j * 512:(j + 1) * 512],
                    start=(c2 == 0), stop=(c2 == NE2 - 1), perf_mode=DR,
                )
        for j in range(2):
            dst = vav[:, i, j * 8:(j + 1) * 8, 0:D]
            src = pv[j].rearrange("p (h d) -> p h d", d=D)
            if g.has_qb:
                vb_view = g.vb_bc.rearrange("p (h d) -> p h d", d=D)[
                    :, j * 8:(j + 1) * 8, :
                ]
                nc.vector.tensor_add(out=dst, in0=src, in1=vb_view)
            else:
                nc.vector.tensor_copy(out=dst, in_=src)


def _emit_ln1_transpose_v(g, xkp, tps, vps):
    """Load x (bf16), LN1, transpose h into e-major hT (fp8); V matmuls are
    interleaved as soon as their h columns are complete."""
    nc = g.nc
    for i in range(NST):
        xt = xkp.tile([P, E], bf16, name="xk")
        nc.sync.dma_start(out=xt, in_=g.xkv_d[i * P:(i + 1) * P, :])
        ht = g.hp.tile([P, E], bf16, name="h")
        _emit_ln(g, xt, ht)
        for c in range(NE):
            # fp8 transposes are illegal; transpose bf16, cast on copy-out
            tp = tps.tile([P, P], bf16, name="tp")
            nc.tensor.transpose(tp, ht[:, c * P:(c + 1) * P], g.ident)
            nc.vector.tensor_copy(out=g.hT[:, c, i * P:(i + 1) * P], in_=tp)
        if i % 2 == 1:
            _emit_v_sp(g, i // 2, vps)


def _emit_qk_unit(g, p, unit, wqkp, qkps):
    """One third of pair p's Q/K projections (unit 0: Q, 1-2: K halves)."""
    nc = g.nc
    w_d = g.wq_d if unit == 0 else g.wk_d
    bias = (g.qb_sb if unit == 0 else g.kb_sb) if g.has_qb else None
    ps = [qkps.tile([P, 512], f32, name=f"ps{j}") for j in range(2)]
    for c2 in range(NE2):
        wsl = wqkp.tile([P, 2, P], f8, name="wsl")
        nc.sync.dma_start(out=wsl, in_=w_d[c2, p])
        for j in range(2):
            s0 = (0 if unit == 0 else (unit - 1) * 1024) + j * 512
            nc.tensor.matmul(
                ps[j], wsl, g.hT[:, 2 * c2:2 * c2 + 2, s0:s0 + 512],
                start=(c2 == 0), stop=(c2 == NE2 - 1), perf_mode=DR,
            )
    for j in range(2):
        if unit == 0:
            dst = g.qt_all[:, p, j * 512:(j + 1) * 512]
        else:
            s0 = (unit - 1) * 1024 + j * 512
            dst = g.kt_all[:, p, s0:s0 + 512]
        if bias is not None:
            nc.vector.tensor_scalar(
                out=dst, in0=ps[j], scalar1=bias[:, p:p + 1], op0=ALU.add
            )
        else:
            nc.vector.tensor_copy(out=dst, in_=ps[j])


def _emit_attn_pair_th(g, p, th, ptp, smp, scps, atps, fillers=()):
    """One query-half of attention for head pair p: scores (transposed), exp
    (fp8), attn^T via DoubleRow over s-pairs, softmax denom from the ones
    column, normalize -> catT.

    `fillers` are emitter thunks injected into the tensor queue between
    s-pair iterations — independent tensor work (next pair's Q/K, MLP up
    chunks) that keeps the tensor engine busy while the in-order scalar
    engine churns through this pair's exps."""
    nc = g.nc
    qt = g.qt_all[:, p, :]
    kt = g.kt_all[:, p, :]
    tcols = slice(th * 512, (th + 1) * 512)
    at0 = atps.tile([VW, 512], f32, name="ps0")
    at1 = atps.tile([VW, 512], f32, name="ps1")
    nfill = len(fillers)
    for sp in range(NSP):
        if nfill and sp % 2 == 1:
            k = (sp - 1) // 2
            if k < nfill:
                fillers[k]()
        pt0 = ptp.tile([P, 2, 512], f8, name="pt0")
        pt1 = ptp.tile([P, 2, 512], f8, name="pt1")
        for i in range(2):
            s = 2 * sp + i
            scols = slice(s * P, (s + 1) * P)
            sc0 = scps.tile([P, 512], f32, name="sc0")
            sc1 = scps.tile([P, 512], f32, name="sc1")
            # S^T[s,t] = (K^T slice).T @ Q^T slice; the two heads live on
            # row-groups 0-63 / 64-127 so the matmuls pack concurrently.
            nc.tensor.matmul(sc0, kt[0:D, scols], qt[0:D, tcols],
                             start=True, stop=True)
            nc.tensor.matmul(sc1, kt[D:2 * D, scols], qt[D:2 * D, tcols],
                             start=True, stop=True)
            nc.scalar.activation(out=pt0[:, i, :], in_=sc0, func=AF.Exp,
                                 scale=EXP_SCALE, bias=g.expb_t)
            nc.scalar.activation(out=pt1[:, i, :], in_=sc1, func=AF.Exp,
                                 scale=EXP_SCALE, bias=g.expb_t)
        nc.tensor.matmul(
            at0, g.va[sp][:, :, (2 * p) * VW:(2 * p) * VW + VW], pt0,
            start=(sp == 0), stop=(sp == NSP - 1), perf_mode=DR,
        )
        nc.tensor.matmul(
            at1, g.va[sp][:, :, (2 * p + 1) * VW:(2 * p + 1) * VW + VW], pt1,
            start=(sp == 0), stop=(sp == NSP - 1), perf_mode=DR,
        )
    se0 = smp.tile([1, 512], f32, name="se0")
    se1 = smp.tile([1, 512], f32, name="se1")
    nc.vector.reciprocal(out=se0, in_=at0[D:D + 1, :])
    nc.vector.reciprocal(out=se1, in_=at1[D:D + 1, :])
    rb0 = smp.tile([D, 512], f32, name="rb0")
    rb1 = smp.tile([D, 512], f32, name="rb1")
    nc.gpsimd.partition_broadcast(rb0, se0)
    nc.gpsimd.partition_broadcast(rb1, se1)
    nc.vector.tensor_mul(out=g.catT[0:D, p, tcols], in0=at0[0:D, :], in1=rb0)
    nc.vector.tensor_mul(out=g.catT[D:2 * D, p, tcols], in0=at1[0:D, :],
                         in1=rb1)


def _emit_proj_ln2_ts(g, ts, xq2p, h2p, pps, t2ps):
    """Proj + residual + LN2 + transpose for one 128-token tile."""
    nc = g.nc
    trows = slice(ts * P, (ts + 1) * P)
    xres = xq2p.tile([P, E], bf16, name="xres")
    nc.sync.dma_start(out=xres, in_=g.xkv_d[ts * P:(ts + 1) * P, :])
    psy = [pps.tile([P, 512], f32, name=f"py{j}") for j in range(2)]
    for c2 in range(NE2):
        for j in range(2):
            nc.tensor.matmul(
                psy[j], g.catT[:, 2 * c2:2 * c2 + 2, trows],
                g.pw_sb[:, c2, :, j * 512:(j + 1) * 512],
                start=(c2 == 0), stop=(c2 == NE2 - 1), perf_mode=DR,
            )
    x2 = g.x2_tiles[ts]
    for j in range(2):
        jc = slice(j * 512, (j + 1) * 512)
        # scalar engine: x2 = psy/256  (fp8 scale compensation)
        nc.scalar.activation(out=x2[:, jc], in_=psy[j], func=AF.Copy,
                             scale=PROJ_OUT_SCALE)
        if g.has_pb:
            nc.vector.tensor_add(out=x2[:, jc], in0=x2[:, jc],
                                 in1=g.pb_bc[:, jc])
        nc.vector.tensor_add(out=x2[:, jc], in0=x2[:, jc], in1=xres[:, jc])
    h2 = h2p.tile([P, E], bf16, name="h2")
    _emit_ln(g, x2, h2)
    for c in range(NE):
        tp = t2ps.tile([P, P], bf16, name="t2")
        nc.tensor.transpose(tp, h2[:, c * P:(c + 1) * P], g.ident)
        nc.vector.tensor_copy(out=g.h2T[c][:, trows], in_=tp)


def _emit_up_f(g, half, f, hid, uwfp, upps):
    """Up-projection + relu for one f-chunk over a 512-token half -> hid."""
    nc = g.nc
    hcols = slice(half * 512, (half + 1) * 512)
    uwf = uwfp.tile([P, NE, P], bf16, name="uwf")
    nc.sync.dma_start(out=uwf, in_=g.uw_d[f])
    pu = upps.tile([P, 512], f32, name="pu")
    for c in range(NE):
        nc.tensor.matmul(
            pu, uwf[:, c, :], g.h2T[c][:, hcols],
            start=(c == 0), stop=(c == NE - 1),
        )
    nc.scalar.activation(out=hid, in_=pu, func=AF.Relu,
                         bias=g.ub_sb[:, f:f + 1])


def _emit_down_fpair(g, f, hid, dns, dwpp):
    """Down-projection accumulation for one f-chunk, two token quarters."""
    nc = g.nc
    dwt = dwpp.tile([P, E], bf16, name="dwt")
    nc.sync.dma_start(out=dwt, in_=g.dw_d[f])
    for qi, dn in enumerate(dns):
        for t2 in range(2):
            for j in range(2):
                nc.tensor.matmul(
                    dn[t2][:, j * 512:(j + 1) * 512],
                    hid[:, qi * 256 + t2 * P:qi * 256 + (t2 + 1) * P],
                    dwt[:, j * 512:(j + 1) * 512],
                    start=(f == 0), stop=(f == NF - 1),
                )


def _emit_mlp_out(g, q, dn, outp):
    nc = g.nc
    for t2 in range(2):
        ti = q * 2 + t2
        ot = outp.tile([P, E], f32, name="ot")
        if g.has_db:
            nc.vector.tensor_add(out=ot, in0=dn[t2], in1=g.db_bc)
            nc.vector.tensor_add(out=ot, in0=ot, in1=g.x2_tiles[ti])
        else:
            nc.vector.tensor_add(out=ot, in0=dn[t2], in1=g.x2_tiles[ti])
        nc.sync.dma_start(out=g.out_d[ti * P:(ti + 1) * P, :], in_=ot)


def _build(flags, reps=1):
    has_qb, has_pb, has_db = flags
    nc = bacc.Bacc("TRN2", target_bir_lowering=False, debug=False, num_devices=8)

    g = _Ctx()
    g.nc = nc
    g.has_qb, g.has_pb, g.has_db = flags
    g.xkv_d = nc.dram_tensor("xkv", [T, E], bf16, kind="ExternalInput")
    g.wq_d = nc.dram_tensor("wq", [NE2, NPAIR, P, 2, P], f8, kind="ExternalInput")
    g.wk_d = nc.dram_tensor("wk", [NE2, NPAIR, P, 2, P], f8, kind="ExternalInput")
    g.wv_d = nc.dram_tensor("wv", [NE2, P, 2, E], f8, kind="ExternalInput")
    g.vrow_d = nc.dram_tensor("vrow", [1, 2 * H * VW], f8, kind="ExternalInput")
    g.pw_d = nc.dram_tensor("pw", [NE2, P, 2, E], f8, kind="ExternalInput")
    g.uw_d = nc.dram_tensor("uw", [NF, P, NE, P], bf16, kind="ExternalInput")
    g.ub_d = nc.dram_tensor("ub", [P, NF], f32, kind="ExternalInput")
    g.dw_d = nc.dram_tensor("dw", [NF, P, E], bf16, kind="ExternalInput")
    if has_qb:
        g.qb_d = nc.dram_tensor("qb", [P, NPAIR], f32, kind="ExternalInput")
        g.kb_d = nc.dram_tensor("kb", [P, NPAIR], f32, kind="ExternalInput")
        g.vbrow_d = nc.dram_tensor("vbrow", [1, E], bf16, kind="ExternalInput")
    if has_pb:
        g.pbrow_d = nc.dram_tensor("pbrow", [1, E], f32, kind="ExternalInput")
    if has_db:
        g.dbrow_d = nc.dram_tensor("dbrow", [1, E], f32, kind="ExternalInput")
    g.out_d = nc.dram_tensor("out", [TQ, E], f32, kind="ExternalOutput")

    with tile.TileContext(nc) as tc:
        with (
            tc.tile_pool(name="consts", bufs=1) as consts,
            tc.tile_pool(name="stat", bufs=4) as stat,
            tc.tile_pool(name="catp", bufs=1) as catp,
            tc.tile_pool(name="x2p", bufs=1) as x2p,
            tc.tile_pool(name="h2Tp", bufs=1) as h2Tp,
        ):
            g.consts, g.stat = consts, stat
            _emit_consts(g)
            for _rep in range(reps):
                _emit_all(g, tc, catp, x2p, h2Tp)

    nc.finalize()
    return nc


def _emit_all(g, tc, catp, x2p, h2Tp):
    nc = g.nc
    g.catT = catp.tile([P, NPAIR, TQ], f8, name="catT")
    g.x2_tiles = [x2p.tile([P, E], bf16, name=f"x2_{i}") for i in range(NTS)]
    g.h2T = [h2Tp.tile([P, TQ], bf16, name=f"h2T{c}") for c in range(NE)]

    g.pwp = tc.alloc_tile_pool(name="pwp", bufs=1)
    g.pw_sb = g.pwp.tile([P, NE2, 2, E], f8, name="pw")
    for c2 in range(NE2):
        nc.gpsimd.dma_start(out=g.pw_sb[:, c2], in_=g.pw_d[c2])

    with (
        tc.tile_pool(name="vaug", bufs=1) as vap,
        tc.tile_pool(name="qkall", bufs=1) as qkallp,
        tc.tile_pool(name="xq2", bufs=3) as xq2p,
        tc.tile_pool(name="h2p", bufs=3) as h2p,
        tc.tile_pool(name="uwfp", bufs=3) as uwfp,
        tc.tile_pool(name="dwpp", bufs=4) as dwpp,
        tc.tile_pool(name="outp", bufs=3) as outp,
        tc.tile_pool(name="ptp", bufs=4) as ptp,
        tc.tile_pool(name="smp", bufs=2) as smp,
    ):
        g.va = [vap.tile([P, 2, H * VW], f8, name=f"va{sp}")
                for sp in range(NSP)]
        g.qt_all = qkallp.tile([P, NPAIR, TQ], f8, name="qtall")
        g.kt_all = qkallp.tile([P, NPAIR, T], f8, name="ktall")

        with (
            tc.tile_pool(name="hp", bufs=4) as hp,
            tc.tile_pool(name="hTp", bufs=1) as hTp,
            tc.tile_pool(name="wvp", bufs=1) as wvp,
        ):
            g.hp = hp
            g.hT = hTp.tile([P, NE, T], f8, name="hT")
            g.wv_sb = wvp.tile([P, NE2, 2, E], f8, name="wv")
            for c2 in range(NE2):
                nc.sync.dma_start(out=g.wv_sb[:, c2], in_=g.wv_d[c2])

            with (
                tc.tile_pool(name="xk", bufs=3) as xkp,
                tc.tile_pool(name="tps", bufs=2, space="PSUM") as tps,
                tc.tile_pool(name="vps", bufs=2, space="PSUM") as vps,
            ):
                _emit_ln1_transpose_v(g, xkp, tps, vps)

            # ---- query half 0 attention; pair p+1's Q/K projections are
            # interleaved into pair p's s-loop as tensor fillers ----
            with (
                tc.tile_pool(name="wqk", bufs=6) as wqkp,
                tc.tile_pool(name="qkps", bufs=1, space="PSUM") as qkps,
                tc.tile_pool(name="atps", bufs=2, space="PSUM") as atps,
                tc.tile_pool(name="scps", bufs=1, space="PSUM") as scps,
            ):
                for u in range(3):
                    _emit_qk_unit(g, 0, u, wqkp, qkps)
                for p in range(NPAIR):
                    if p + 1 < NPAIR:
                        fillers = [
                            (lambda u=u, pp=p + 1:
                             _emit_qk_unit(g, pp, u, wqkp, qkps))
                            for u in range(3)
                        ]
                    else:
                        fillers = []
                    _emit_attn_pair_th(g, p, 0, ptp, smp, scps, atps,
                                       fillers=fillers)

        # ---- proj + LN2 for token tiles 0-3 ----
        with (
            tc.tile_pool(name="pps", bufs=2, space="PSUM") as pps,
            tc.tile_pool(name="t2ps", bufs=2, space="PSUM") as t2ps,
        ):
            for ts in range(4):
                _emit_proj_ln2_ts(g, ts, xq2p, h2p, pps, t2ps)

        # ---- query half 1 attention, interleaved with MLP up for token
        # half 0 (tensor filler while the scalar engine does exps) ----
        with tc.tile_pool(name="hid0p", bufs=1) as hid0p:
            g.hid0 = [hid0p.tile([P, 512], bf16, name=f"hid0_{f}")
                      for f in range(NF)]
            with (
                tc.tile_pool(name="atps", bufs=2, space="PSUM") as atps,
                tc.tile_pool(name="scps", bufs=1, space="PSUM") as scps,
                tc.tile_pool(name="upps", bufs=2, space="PSUM") as upps,
            ):
                for p in range(NPAIR):
                    fillers = [
                        (lambda f=f: _emit_up_f(g, 0, f, g.hid0[f], uwfp,
                                                upps))
                        for f in range(4 * p, 4 * p + 4)
                    ]
                    _emit_attn_pair_th(g, p, 1, ptp, smp, scps, atps,
                                       fillers=fillers)

            # ---- proj + LN2 for token tiles 4-7 ----
            with (
                tc.tile_pool(name="pps", bufs=2, space="PSUM") as pps,
                tc.tile_pool(name="t2ps", bufs=2, space="PSUM") as t2ps,
            ):
                for ts in range(4, 8):
                    _emit_proj_ln2_ts(g, ts, xq2p, h2p, pps, t2ps)

            # ---- down for token quarters 0+1 (dw streamed once) ----
            with tc.tile_pool(name="dnps", bufs=1, space="PSUM") as dnps:
                dn0 = [dnps.tile([P, E], f32, name=f"dn0{j}") for j in range(2)]
                dn1 = [dnps.tile([P, E], f32, name=f"dn1{j}") for j in range(2)]
                for f in range(NF):
                    _emit_down_fpair(g, f, g.hid0[f], [dn0, dn1], dwpp)
                _emit_mlp_out(g, 0, dn0, outp)
                _emit_mlp_out(g, 1, dn1, outp)

        # ---- up for token half 1, then down for quarters 2+3 ----
        with tc.tile_pool(name="hid1p", bufs=1) as hid1p:
            g.hid1 = [hid1p.tile([P, 512], bf16, name=f"hid1_{f}")
                      for f in range(NF)]
            with tc.tile_pool(name="upps", bufs=3, space="PSUM") as upps:
                for f in range(NF):
                    _emit_up_f(g, 1, f, g.hid1[f], uwfp, upps)
            with tc.tile_pool(name="dnps", bufs=1, space="PSUM") as dnps:
                dn2 = [dnps.tile([P, E], f32, name=f"dn0{j}") for j in range(2)]
                dn3 = [dnps.tile([P, E], f32, name=f"dn1{j}") for j in range(2)]
                for f in range(NF):
                    _emit_down_fpair(g, f, g.hid1[f], [dn2, dn3], dwpp)
                _emit_mlp_out(g, 2, dn2, outp)
                _emit_mlp_out(g, 3, dn3, outp)
    g.pwp.release()


def _get_nc(flags, reps=1):
    key = (flags, reps)
    if key not in _BUILD_CACHE:
        _BUILD_CACHE[key] = _build(flags, reps)
    return _BUILD_CACHE[key]


def _prep(x, Wq, Wk, Wv, proj_w, proj_b, ln1_g, ln1_b, ln2_g, ln2_b,
          up_w, up_b, down_w, down_b):
    """Host-side shard + weight fold/cast/layout. Returns (flags, in_maps)."""
    bfl = ml_dtypes.bfloat16
    f8l = ml_dtypes.float8_e4m3fn
    x = np.asarray(x, dtype=np.float32)
    Wq = np.asarray(Wq, np.float32)
    Wk = np.asarray(Wk, np.float32)
    Wv = np.asarray(Wv, np.float32)
    g1 = np.asarray(ln1_g, np.float32)
    b1 = np.asarray(ln1_b, np.float32)
    g2 = np.asarray(ln2_g, np.float32)
    b2 = np.asarray(ln2_b, np.float32)
    proj_w = np.asarray(proj_w, np.float32)
    up_w = np.asarray(up_w, np.float32)
    down_w = np.asarray(down_w, np.float32)

    # [H, E, D] -> [E, H*D]; fold LN1 gain + fp8 range scales into weights.
    # (1/sqrt(D) and the QS^2 compensation fold into the Exp activation.)
    wq_all = Wq.transpose(1, 0, 2).reshape(E, E) * QS
    wk_all = Wk.transpose(1, 0, 2).reshape(E, E) * QS
    wv_all = Wv.transpose(1, 0, 2).reshape(E, E) * VS
    qb_vec = b1 @ wq_all
    kb_vec = b1 @ wk_all
    vb_vec = b1 @ wv_all
    wq_f = g1[:, None] * wq_all
    wk_f = g1[:, None] * wk_all
    wv_f = g1[:, None] * wv_all

    def _qk_chunks(w):  # [E, E] -> [NE2, NPAIR, P, 2, P]
        return np.ascontiguousarray(
            w.reshape(NE2, 2, P, NPAIR, P).transpose(0, 3, 2, 1, 4).astype(f8l)
        )

    def _row_chunks(w):  # [E, N] -> [NE2, P, 2, N]
        n = w.shape[1]
        return np.ascontiguousarray(
            w.reshape(NE2, 2, P, n).transpose(0, 2, 1, 3).astype(f8l)
        )

    vrow = np.zeros((1, 2 * H * VW), np.float32)
    vrow.reshape(2, H, VW)[:, :, D] = 1.0

    uw_f = g2[:, None] * up_w
    ub_f = np.asarray(up_b, np.float32) + b2 @ up_w

    has_qb = bool(np.any(b1 != 0))
    has_pb = bool(np.any(np.asarray(proj_b) != 0))
    has_db = bool(np.any(np.asarray(down_b) != 0))
    flags = (has_qb, has_pb, has_db)

    shared = {
        "wq": _qk_chunks(wq_f),
        "wk": _qk_chunks(wk_f),
        "wv": _row_chunks(wv_f),
        "vrow": vrow.astype(f8l),
        "pw": _row_chunks(proj_w * PS),
        # uw as [NF, P, NE, P]: per-f tile holding all 8 e-chunks
        "uw": np.ascontiguousarray(
            uw_f.reshape(NE, P, NF, P).transpose(2, 1, 0, 3).astype(bfl)),
        "ub": np.ascontiguousarray(ub_f.reshape(NF, P).T.astype(np.float32)),
        "dw": np.ascontiguousarray(down_w.reshape(NF, P, E).astype(bfl)),
    }
    if has_qb:
        shared["qb"] = np.ascontiguousarray(
            qb_vec.reshape(NPAIR, P).T.astype(np.float32))
        shared["kb"] = np.ascontiguousarray(
            kb_vec.reshape(NPAIR, P).T.astype(np.float32))
        shared["vbrow"] = vb_vec.reshape(1, E).astype(bfl)
    if has_pb:
        shared["pbrow"] = np.asarray(proj_b, np.float32).reshape(1, E)
    if has_db:
        shared["dbrow"] = np.asarray(down_b, np.float32).reshape(1, E)

    in_maps = []
    for c in range(8):
        b, half = c // 2, c % 2
        xb = x[b]
        if half == 1:
            xb = np.concatenate([xb[TQ:], xb[:TQ]], axis=0)
        in_maps.append({"xkv": np.ascontiguousarray(xb.astype(bfl)), **shared})
    return flags, in_maps


def kernel(**inputs) -> np.ndarray:
    flags, in_maps = _prep(**inputs)
    nc = _get_nc(flags)
    res = run_bass_kernel_spmd(nc, in_maps, core_ids=list(range(8)))
    out = np.empty((B, T, E), np.float32)
    for c in range(8):
        b, half = c // 2, c % 2
        out[b, half * TQ:(half + 1) * TQ, :] = res.results[c]["out"]
    return out
